# revision 16
# baseline (speedup 1.0000x reference)
"""Conv3x3(8->64) + GroupNorm(16) + scale + MaxPool4 + clamp kernel for TRN2.

v5 layout (per core, S samples). All APs <= 3 free dims (walrus limit).
  - i4[128, 3972]: partition (32g + 4ic + r) holds x[n, ic, 32g+r ...]
    (row-shifted quartering, j-trick K=32); 4 per-quarter HWDGE DMAs,
    uniform length thanks to 512-elem x padding.
  - conv: 8 PSUM half-tiles [128, 1024]/sample (pool bufs=4 = all 8
    banks); per half-tile, 6 matmuls issued kw-outer/g-inner
    (tile_position=(32g,0)). Fine tiling keeps ACT drain fully hidden
    behind the PE stream (HW-ablated: PE is the bottleneck ~297ns/MM).
  - PSUM col = 128*pr + w (plain 2-dim rhs AP: measurably faster PE
    streaming than the parity-split 3-dim AP): p1 is step-2 1x, p2 2x,
    p3 step-2 1x (DVE has slack; hidden under the PE wall). Pads
    (w 126/127) at the end of each 128-block.
  - ybf blocks in Q = 16t+4g+pr order, so each tile drains with ONE
    3-dim ACT (Identity+bias, accum_out = exact sum(y), pad-free reads).
  - sumsq: split DVE scalar_tensor_tensor (1x) / ACT Square+accum.
  - fold: partition fold via contiguous SBUF DMA + p4 max; the h-quad
    reorder (block u'=8t+2g+c -> i=8g+2t+c) rides the split clamp-cast
    (f16->f32), keeping the out DMA contiguous 3.8KB/partition.
  - stats: SEL matmul into spare PSUM cols of tile3; fused small-op
    chain (Sqrt scale=ws^2 yields A directly; B computed negated).
"""

import numpy as np
import concourse.bass as bass
import concourse.tile as tile
from concourse import bacc, mybir
from contextlib import ExitStack

F32 = mybir.dt.float32
BF16 = mybir.dt.bfloat16
F16 = mybir.dt.float16

EPS = 1e-5
NPIX = 4 * 126 * 126  # elements per (sample, group)
XELEM = 8 * 128 * 128  # elements per sample of x
# DVE share of the sumsq pass, in 128-col ybf blocks (of 63 total).
SQ_DVE_BLOCKS = 38


def _ap(base, dims, offset):
    """Copy of AP `base` with raw [step,count] dims and element offset."""
    a = base.copy()
    a.ap = mybir.VecI64Pair([list(d) for d in dims])
    a.offset = offset
    return a


def _apf(base, free_dims, elem_offset):
    """SBUF AP: keep `base`'s partition dim, replace free dims, add offset."""
    a = base.copy()
    a.ap = mybir.VecI64Pair([list(base.ap[0])] + [list(d) for d in free_dims])
    a.offset = base.offset + elem_offset
    return a


class Pools:
    pass


def build_pools(ctx, tc):
    p = Pools()
    p.consts = ctx.enter_context(tc.tile_pool(name="consts", bufs=1))
    p.i4 = ctx.enter_context(tc.tile_pool(name="i4", bufs=3))
    p.ps = ctx.enter_context(tc.tile_pool(name="psc", bufs=4, space="PSUM"))
    p.y = ctx.enter_context(tc.tile_pool(name="ybuf", bufs=2))
    p.sq = ctx.enter_context(tc.tile_pool(name="sqbuf", bufs=2))
    p.pool = ctx.enter_context(tc.tile_pool(name="pools", bufs=2))
    p.st = ctx.enter_context(tc.tile_pool(name="stats", bufs=2))
    return p


def load_consts(nc, p, wq_in, sel_in, bias_in, ws_in, gb_in):
    c = Pools()
    c.wq64 = p.consts.tile([128, 128], BF16, tag="wq64")
    nc.sync.dma_start(c.wq64[:], wq_in[0])
    c.wq32 = p.consts.tile([128, 128], BF16, tag="wq32")
    nc.sync.dma_start(c.wq32[:], wq_in[1])
    c.sel = p.consts.tile([128, 64], F32, tag="sel")
    nc.sync.dma_start(c.sel[:], sel_in[:])
    c.cbias = p.consts.tile([128, 1], F32, tag="cbias")
    nc.sync.dma_start(c.cbias[:], bias_in[:])
    c.ws2 = p.consts.tile([64, 1], F32, tag="ws2")
    nc.sync.dma_start(c.ws2[:], ws_in[:])
    c.gb = p.consts.tile([64, 1], F32, tag="gb")
    nc.sync.dma_start(c.gb[:], gb_in[:])
    return c


def sample_prefetch(nc, p, x_in, n):
    """i4 per-quarter DMAs (DMA APs cap at 3 dims); uniform length thanks
    to the 512-elem x padding. Issued one sample ahead so the SP-queue
    waits of phase2 DMAs never delay the next sample's input load."""
    tiles = []
    for half in range(2):
        i4 = p.i4.tile([128, 3972], BF16, tag="i4")
        tiles.append(i4)
        base = i4[:]
        pstep = base.ap[0][0]
        for gg in range(2):
            g = 2 * half + gg
            for kwh in range(2):
                dst = _ap(base, [[pstep, 32], [1, 3972]],
                          base.offset + (64 * gg + 32 * kwh) * pstep)
                src = _ap(x_in, [[16384, 8], [128, 4], [1, 3972]],
                          n * XELEM + 4096 * g + kwh)
                nc.sync.dma_start(dst, src)
    st = Pools()
    st.i4 = tiles
    return st


def sample_phase1(nc, tc, p, c, st1, n):
    """DMA + conv + ACT copies + pools to p3 + sumsq + stat2.

    Returns state consumed by sample_phase2, which is issued after the
    NEXT sample's phase1 so that cross-sample waits (Sqrt on ACT, SEL
    matmul on PE, fold/out DMA on SP) never block the next sample's
    work at the head of an engine FIFO.
    """
    AL = mybir.AluOpType
    AF = mybir.ActivationFunctionType

    i4 = st1.i4
    ybf = p.y.tile([128, 8064], F16, tag="ybf")
    sacc = p.st.tile([128, 8], F32, tag="sacc")
    ssq = p.st.tile([128, 2], F32, tag="ssq")

    # --- conv: 4 tiles x (3 kw x 4 g) matmuls, g-inner for PE row-group
    # concurrency. rhs free dims (pr, a, m): col = 1024t+256pr+kw+a+2m.
    tiles = []
    for t in range(4):
        psa = p.ps.tile([128, 1024], F32, tag="ps")
        psb = p.ps.tile([128, 1024], F32, tag="ps")
        tiles += [psa, psb]
        # kw 0,1 folded into the contraction (K=64: partitions (kwh,ic,t'));
        # kw=2 accumulated via a K=32 matmul on the kwh=0 sub-rows at
        # free-offset +2. 2 MMs per (t,g) instead of 3.
        for step in range(2):
            for g in range(4):
                ps, gl = (psa, g) if g < 2 else (psb, g - 2)
                ia = i4[g // 2]
                prow = 64 * (g % 2)
                npair = 3 if (t == 3 and g == 3) else 4
                out = ps[:, 512 * gl: 512 * gl + 128 * npair]
                if step == 0:
                    rhs = _apf(ia[prow: prow + 64],
                               [[256, npair], [1, 128]], 1024 * t)
                    nc.tensor.matmul(out, c.wq64[prow: prow + 64, :], rhs,
                                     start=True, stop=False,
                                     tile_position=(prow, 0))
                else:
                    rhs = _apf(ia[prow: prow + 32],
                               [[256, npair], [1, 128]], 1024 * t + 2)
                    nc.tensor.matmul(out, c.wq32[prow: prow + 32, :], rhs,
                                     start=False, stop=True,
                                     tile_position=(prow, 0))

    # --- ACT copies: pad-free reads, accum = exact per-partition sum(y).
    # ybf block Q = 16t + 4g + pr at offset 128Q (tile-major: one ACT/tile).
    for u in range(8):
        ps = tiles[u]
        nblk = 8 if u < 7 else 7
        src = _apf(ps[:], [[128, nblk], [1, 126]], 0)
        dst = _apf(ybf[:], [[128, nblk], [1, 126]], 1024 * u)
        nc.scalar.activation(dst, src, AF.Identity, bias=c.cbias[:, 0:1],
                             scale=1.0, accum_out=sacc[:, u:u + 1])

    # zero the pad cols (63, 127 of each 128-block) so sumsq stays exact
    nc.gpsimd.memset(_apf(ybf[:], [[128, 63], [1, 2]], 126), 0.0)

    # --- pools (step-1 f16 TT at 2x except the step-2 p3)
    p1 = p.pool.tile([128, 4032], F16, tag="p1")
    nc.vector.tensor_tensor(p1[:], _apf(ybf[:], [[128, 63], [2, 64]], 0),
                            _apf(ybf[:], [[128, 63], [2, 64]], 1), op=AL.max)

    p2 = p.pool.tile([128, 1984], F16, tag="p2")
    nc.vector.tensor_tensor(p2[:], _apf(p1[:], [[128, 31], [1, 64]], 0),
                            _apf(p1[:], [[128, 31], [1, 64]], 64), op=AL.max)

    p3 = p.pool.tile([128, 961], F16, tag="p3")
    nc.vector.tensor_tensor(p3[:], _apf(p2[:], [[64, 31], [2, 31]], 0),
                            _apf(p2[:], [[64, 31], [2, 31]], 1), op=AL.max)

    # --- sumsq, split DVE / ACT; squares land in sqbuf (junk)
    LD = 128 * SQ_DVE_BLOCKS
    sqb = p.sq.tile([128, 8064], F16, tag="sqb")
    yv = _apf(ybf[:], [[1, LD]], 0)
    nc.vector.scalar_tensor_tensor(_apf(sqb[:], [[1, LD]], 0), yv, 1.0, yv,
                                   op0=AL.mult, op1=AL.mult,
                                   accum_out=ssq[:, 0:1])
    nc.scalar.activation(_apf(sqb[:], [[1, 8064 - LD]], LD),
                         _apf(ybf[:], [[1, 8064 - LD]], LD), AF.Square,
                         accum_out=ssq[:, 1:2])

    stat2 = p.st.tile([128, 2], F32, tag="stat2")
    nc.vector.reduce_sum(stat2[:, 0:1], sacc[:], axis=mybir.AxisListType.X)
    nc.vector.tensor_tensor(stat2[:, 1:2], ssq[:, 0:1], ssq[:, 1:2], op=AL.add)

    st = Pools()
    st.p3, st.stat2, st.gsum = p3, stat2, tiles[7][0:64, 960:962]
    return st


def sample_phase2(nc, tc, p, c, st, y_out, n):
    """Stats tail + partition fold + finalize + output for sample n."""
    AL = mybir.AluOpType
    AF = mybir.ActivationFunctionType

    # group fold via SEL matmul into spare PSUM cols (never conv-written)
    nc.tensor.matmul(st.gsum, c.sel[:], st.stat2[:], start=True, stop=True)

    # partition fold: remap upper half down via DMA, then max
    p3b = p.pool.tile([64, 961], F16, tag="p3b")
    nc.sync.dma_start(p3b[:], st.p3[64:128, :])

    mv = p.st.tile([64, 2], F32, tag="mv")
    nc.vector.tensor_scalar(mv[:], st.gsum, 1.0 / NPIX, None, op0=AL.mult)
    msq = p.st.tile([64, 1], F32, tag="msq")
    nc.vector.tensor_tensor(msq[:], mv[:, 0:1], mv[:, 0:1], op=AL.mult)
    veps = p.st.tile([64, 1], F32, tag="veps")
    nc.vector.scalar_tensor_tensor(veps[:], mv[:, 1:2], EPS, msq[:],
                                   op0=AL.add, op1=AL.subtract)
    rv = p.st.tile([64, 1], F32, tag="rv")
    nc.vector.reciprocal(rv[:], veps[:])
    # aap = sqrt(rv * ws^2) = ws/std  (ws > 0, asserted host-side)
    aap = p.st.tile([64, 1], F32, tag="aap")
    nc.scalar.activation(aap[:], rv[:], AF.Sqrt, scale=c.ws2[:, 0:1])
    # bapneg = mean*aap - gb  (so fin = p4*aap - bapneg)
    bapneg = p.st.tile([64, 1], F32, tag="bapneg")
    nc.vector.scalar_tensor_tensor(bapneg[:], mv[:, 0:1], aap[:, 0:1],
                                   c.gb[:], op0=AL.mult, op1=AL.subtract)

    p4 = p.pool.tile([64, 961], F16, tag="p4")
    nc.vector.tensor_tensor(p4[:], st.p3[0:64, :], p3b[:], op=AL.max)

    # --- finalize: A*p - Bneg (f16, 4x), then split clamp-cast to f32 that
    # also reorders blocks u' = 8t+2g+c -> i = 8g+2t+c for a contiguous
    # out DMA (f32 writes stay 4B-aligned at odd 31-col offsets).
    fin = p.pool.tile([64, 961], F16, tag="fin")
    nc.vector.tensor_scalar(fin[:], p4[:], aap[:, 0:1], bapneg[:, 0:1],
                            op0=AL.mult, op1=AL.subtract)
    fin32 = p.pool.tile([64, 961], F32, tag="fin32")
    for sd, dd, off_s, off_d in (
            ([[248, 4], [62, 4], [1, 31]], [[62, 4], [248, 4], [1, 31]], 0, 0),
            ([[248, 3], [62, 4], [1, 31]], [[62, 3], [248, 4], [1, 31]], 31, 31),
            ([[62, 3], [1, 31]], [[248, 3], [1, 31]], 775, 217)):
        nc.vector.tensor_scalar(_apf(fin32[:], dd, off_d),
                                _apf(fin[:], sd, off_s),
                                0.0, 1.0, op0=AL.max, op1=AL.min)

    dst = _ap(y_out, [[961, 64], [1, 961]], n * 61504)
    nc.sync.dma_start(dst, fin32[:])


def build_kernel_nc(S, n_cores=8, repeat=1, use_for_i=False):
    nc = bacc.Bacc("TRN2", target_bir_lowering=False, debug=False,
                   num_devices=n_cores)
    x_in = nc.dram_tensor("x", [S * XELEM + 512], BF16,
                          kind="ExternalInput").ap()
    wq_in = nc.dram_tensor("wq", [2, 128, 128], BF16,
                          kind="ExternalInput").ap()
    sel_in = nc.dram_tensor("sel", [128, 64], F32, kind="ExternalInput").ap()
    bias_in = nc.dram_tensor("cbias", [128, 1], F32, kind="ExternalInput").ap()
    ws_in = nc.dram_tensor("ws", [64, 1], F32, kind="ExternalInput").ap()
    gb_in = nc.dram_tensor("gb", [64, 1], F32, kind="ExternalInput").ap()
    y_out = nc.dram_tensor("y", [S, 64, 31, 31], F32, kind="ExternalOutput").ap()
    with tile.TileContext(nc) as tc:
        with ExitStack() as ctx:
            p = build_pools(ctx, tc)
            c = load_consts(nc, p, wq_in, sel_in, bias_in, ws_in, gb_in)
            def pipelined_batch():
                pf = sample_prefetch(nc, p, x_in, 0)
                prev = None
                for n in range(S):
                    pf_next = (sample_prefetch(nc, p, x_in, n + 1)
                               if n + 1 < S else None)
                    if prev is not None:
                        sample_phase2(nc, tc, p, c, prev, y_out, n - 1)
                    prev = sample_phase1(nc, tc, p, c, pf, n)
                    pf = pf_next
                sample_phase2(nc, tc, p, c, prev, y_out, S - 1)

            if use_for_i and repeat > 1:
                with tc.For_i(0, repeat, 1):
                    pipelined_batch()
            else:
                for _ in range(repeat):
                    pipelined_batch()
    nc.compile()
    return nc


def make_consts(conv_w, conv_b, gn_w, gn_b, scale):
    """Host-side constant assembly."""
    import ml_dtypes
    conv_w = np.asarray(conv_w, dtype=np.float32)
    # w64[32*kwh + 4*ic + (kh+j), oc+64j] = conv_w[oc, ic, kh, kwh]
    # w32[4*ic + (kh+j), oc+64j] = conv_w[oc, ic, kh, 2]
    oc = np.arange(64)
    w64 = np.zeros((64, 128), np.float32)
    w32 = np.zeros((32, 128), np.float32)
    for j in range(2):
        for ic in range(8):
            for kh in range(3):
                for kwh in range(2):
                    w64[32 * kwh + ic * 4 + kh + j, oc + 64 * j] = \
                        conv_w[oc, ic, kh, kwh]
                w32[ic * 4 + kh + j, oc + 64 * j] = conv_w[oc, ic, kh, 2]
    wq64 = np.tile(w64, (2, 1))
    wq32 = np.zeros((128, 128), np.float32)
    wq32[0:32] = w32
    wq32[64:96] = w32
    wq = np.stack([wq64, wq32]).astype(ml_dtypes.bfloat16)
    sel = np.zeros((128, 64), np.float32)
    for j in range(2):
        for o in range(64):
            sel[o + 64 * j, (o // 4) * 4: (o // 4) * 4 + 4] = 1.0
    cbias = np.tile(np.asarray(conv_b, np.float32).reshape(64, 1), (2, 1))
    ws = (np.asarray(gn_w, np.float32).reshape(64) *
          np.asarray(scale, np.float32).reshape(64)).reshape(64, 1)
    assert np.all(ws > 0), "sqrt(rv*ws^2) trick needs ws > 0"
    gb = (np.asarray(gn_b, np.float32).reshape(64) *
          np.asarray(scale, np.float32).reshape(64)).reshape(64, 1)
    return dict(wq=wq, sel=sel, cbias=cbias.astype(np.float32),
                ws=(ws * ws).astype(np.float32), gb=gb.astype(np.float32))


# ---------------------------------------------------------------------------
# Harness entry point: full (unsharded) inputs -> full output.
# ---------------------------------------------------------------------------
N_CORES = 8
S_PER_CORE = 16
_NC_CACHE = {}


def _get_nc(repeat=1, use_for_i=False):
    key = (repeat, use_for_i)
    if key not in _NC_CACHE:
        _NC_CACHE[key] = build_kernel_nc(S_PER_CORE, n_cores=N_CORES,
                                         repeat=repeat, use_for_i=use_for_i)
    return _NC_CACHE[key]


def cast_x(x):
    import ml_dtypes
    return np.ascontiguousarray(np.asarray(x, dtype=np.float32)).astype(
        ml_dtypes.bfloat16)


def shard_x(xb, core):
    """Per-core padded flat x slice."""
    flat = xb[core * S_PER_CORE:(core + 1) * S_PER_CORE].reshape(-1)
    return np.concatenate([flat, np.zeros(512, dtype=flat.dtype)])


def kernel(x, conv_w, conv_b, gn_w, gn_b, scale):
    from concourse.bass_utils import run_bass_kernel_spmd
    xb = cast_x(x)
    consts = make_consts(conv_w, conv_b, gn_w, gn_b, scale)
    nc = _get_nc()
    in_maps = []
    for cc in range(N_CORES):
        m = dict(consts)
        m["x"] = shard_x(xb, cc)
        in_maps.append(m)
    res = run_bass_kernel_spmd(nc, in_maps, core_ids=list(range(N_CORES)))
    return np.concatenate([res.results[cc]["y"] for cc in range(N_CORES)],
                          axis=0)


# revision 18
# speedup vs baseline: 1.7116x; 1.7116x over previous
"""Conv3x3(8->64) + GroupNorm(16) + scale + MaxPool4 + clamp kernel for TRN2.

v5 layout (per core, S samples). All APs <= 3 free dims (walrus limit).
  - i4[128, 3972]: partition (32g + 4ic + r) holds x[n, ic, 32g+r ...]
    (row-shifted quartering, j-trick K=32); 4 per-quarter HWDGE DMAs,
    uniform length thanks to 512-elem x padding.
  - conv: 8 PSUM half-tiles [128, 1024]/sample (pool bufs=4 = all 8
    banks); per half-tile, 6 matmuls issued kw-outer/g-inner
    (tile_position=(32g,0)). Fine tiling keeps ACT drain fully hidden
    behind the PE stream (HW-ablated: PE is the bottleneck ~297ns/MM).
  - PSUM col = 128*pr + w (plain 2-dim rhs AP: measurably faster PE
    streaming than the parity-split 3-dim AP): p1 is step-2 1x, p2 2x,
    p3 step-2 1x (DVE has slack; hidden under the PE wall). Pads
    (w 126/127) at the end of each 128-block.
  - ybf blocks in Q = 16t+4g+pr order, so each tile drains with ONE
    3-dim ACT (Identity+bias, accum_out = exact sum(y), pad-free reads).
  - sumsq: split DVE scalar_tensor_tensor (1x) / ACT Square+accum.
  - fold: partition fold via contiguous SBUF DMA + p4 max; the h-quad
    reorder (block u'=8t+2g+c -> i=8g+2t+c) rides the split clamp-cast
    (f16->f32), keeping the out DMA contiguous 3.8KB/partition.
  - stats: SEL matmul into spare PSUM cols of tile3; fused small-op
    chain (Sqrt scale=ws^2 yields A directly; B computed negated).
"""

import numpy as np
import concourse.bass as bass
import concourse.tile as tile
from concourse import bacc, mybir
from contextlib import ExitStack

F32 = mybir.dt.float32
BF16 = mybir.dt.bfloat16
F16 = mybir.dt.float16

EPS = 1e-5
NPIX = 4 * 126 * 126  # elements per (sample, group)
XELEM = 8 * 128 * 128  # elements per sample of x
# DVE share of the sumsq pass, in 128-col ybf blocks (of 63 total).
SQ_DVE_BLOCKS = 38


def _ap(base, dims, offset):
    """Copy of AP `base` with raw [step,count] dims and element offset."""
    a = base.copy()
    a.ap = mybir.VecI64Pair([list(d) for d in dims])
    a.offset = offset
    return a


def _apf(base, free_dims, elem_offset):
    """SBUF AP: keep `base`'s partition dim, replace free dims, add offset."""
    a = base.copy()
    a.ap = mybir.VecI64Pair([list(base.ap[0])] + [list(d) for d in free_dims])
    a.offset = base.offset + elem_offset
    return a


class Pools:
    pass


def build_pools(ctx, tc):
    p = Pools()
    p.consts = ctx.enter_context(tc.tile_pool(name="consts", bufs=1))
    p.i4 = ctx.enter_context(tc.tile_pool(name="i4", bufs=3))
    p.ps = ctx.enter_context(tc.tile_pool(name="psc", bufs=4, space="PSUM"))
    p.y = ctx.enter_context(tc.tile_pool(name="ybuf", bufs=2))
    p.sq = ctx.enter_context(tc.tile_pool(name="sqbuf", bufs=2))
    p.pool = ctx.enter_context(tc.tile_pool(name="pools", bufs=2))
    p.st = ctx.enter_context(tc.tile_pool(name="stats", bufs=2))
    return p


def load_consts(nc, p, wq_in, sel_in, bias_in, ws_in, gb_in):
    c = Pools()
    c.wq = p.consts.tile([128, 384], BF16, tag="wq")
    nc.sync.dma_start(c.wq[:], wq_in[:])
    c.sel = p.consts.tile([128, 64], F32, tag="sel")
    nc.sync.dma_start(c.sel[:], sel_in[:])
    c.cbias = p.consts.tile([128, 1], F32, tag="cbias")
    nc.sync.dma_start(c.cbias[:], bias_in[:])
    c.ws2 = p.consts.tile([64, 1], F32, tag="ws2")
    nc.sync.dma_start(c.ws2[:], ws_in[:])
    c.gb = p.consts.tile([64, 1], F32, tag="gb")
    nc.sync.dma_start(c.gb[:], gb_in[:])
    return c


def sample_prefetch(nc, p, x_in, n):
    """i4 per-quarter DMAs (DMA APs cap at 3 dims); uniform length thanks
    to the 512-elem x padding. Issued one sample ahead so the SP-queue
    waits of phase2 DMAs never delay the next sample's input load."""
    i4 = p.i4.tile([128, 3972], BF16, tag="i4")
    base = i4[:]
    pstep = base.ap[0][0]
    for g in range(4):
        dst = _ap(base, [[pstep, 32], [1, 3972]],
                  base.offset + 32 * g * pstep)
        src = _ap(x_in, [[16384, 8], [128, 4], [1, 3972]],
                  n * XELEM + 4096 * g)
        nc.sync.dma_start(dst, src)
    st = Pools()
    st.i4 = i4
    return st


def sample_phase1(nc, tc, p, c, st1, n):
    """DMA + conv + ACT copies + pools to p3 + sumsq + stat2.

    Returns state consumed by sample_phase2, which is issued after the
    NEXT sample's phase1 so that cross-sample waits (Sqrt on ACT, SEL
    matmul on PE, fold/out DMA on SP) never block the next sample's
    work at the head of an engine FIFO.
    """
    AL = mybir.AluOpType
    AF = mybir.ActivationFunctionType

    i4 = st1.i4
    ybf = p.y.tile([128, 8064], F16, tag="ybf")
    sacc = p.st.tile([128, 8], F32, tag="sacc")
    ssq = p.st.tile([128, 2], F32, tag="ssq")

    # --- conv: 4 tiles x (3 kw x 4 g) matmuls, g-inner for PE row-group
    # concurrency. rhs free dims (pr, a, m): col = 1024t+256pr+kw+a+2m.
    tiles = []
    for t in range(4):
        psa = p.ps.tile([128, 1024], F32, tag="ps")
        psb = p.ps.tile([128, 1024], F32, tag="ps")
        tiles += [psa, psb]
        # rotate over all 4 PE row-groups so each group has ~3 MM slots of
        # idle before its next LDWEIGHTS (same-group LDW must wait for the
        # group's previous MM to finish streaming)
        # g-outer / kw-inner: consecutive MMs hit the same row-group, so
        # each next LDWEIGHTS can prefetch into the PE's background weight
        # buffer while the group streams.
        for g in range(4):
            for kw in range(3):
                ps, gl = (psa, g) if g < 2 else (psb, g - 2)
                npair = 3 if (t == 3 and g == 3) else 4
                wk = c.wq[32 * g: 32 * g + 32, 128 * kw: 128 * kw + 128]
                rhs = _apf(i4[32 * g: 32 * g + 32],
                           [[256, npair], [1, 128]],
                           1024 * t + kw)
                nc.tensor.matmul(
                    ps[:, 512 * gl: 512 * gl + 128 * npair],
                    wk, rhs, start=(kw == 0), stop=(kw == 2),
                    tile_position=(32 * g, 0))

    # --- ACT copies: pad-free reads, accum = exact per-partition sum(y).
    # ybf block Q = 16t + 4g + pr at offset 128Q (tile-major: one ACT/tile).
    for u in range(8):
        ps = tiles[u]
        nblk = 8 if u < 7 else 7
        src = _apf(ps[:], [[128, nblk], [1, 126]], 0)
        dst = _apf(ybf[:], [[128, nblk], [1, 126]], 1024 * u)
        nc.scalar.activation(dst, src, AF.Identity, bias=c.cbias[:, 0:1],
                             scale=1.0, accum_out=sacc[:, u:u + 1])

    # zero the pad cols (63, 127 of each 128-block) so sumsq stays exact
    nc.gpsimd.memset(_apf(ybf[:], [[128, 63], [1, 2]], 126), 0.0)

    # --- pools (step-1 f16 TT at 2x except the step-2 p3)
    p1 = p.pool.tile([128, 4032], F16, tag="p1")
    nc.vector.tensor_tensor(p1[:], _apf(ybf[:], [[128, 63], [2, 64]], 0),
                            _apf(ybf[:], [[128, 63], [2, 64]], 1), op=AL.max)

    p2 = p.pool.tile([128, 1984], F16, tag="p2")
    nc.vector.tensor_tensor(p2[:], _apf(p1[:], [[128, 31], [1, 64]], 0),
                            _apf(p1[:], [[128, 31], [1, 64]], 64), op=AL.max)

    p3 = p.pool.tile([128, 961], F16, tag="p3")
    nc.vector.tensor_tensor(p3[:], _apf(p2[:], [[64, 31], [2, 31]], 0),
                            _apf(p2[:], [[64, 31], [2, 31]], 1), op=AL.max)

    # --- sumsq, split DVE / ACT; squares land in sqbuf (junk)
    LD = 128 * SQ_DVE_BLOCKS
    sqb = p.sq.tile([128, 8064], F16, tag="sqb")
    yv = _apf(ybf[:], [[1, LD]], 0)
    nc.vector.scalar_tensor_tensor(_apf(sqb[:], [[1, LD]], 0), yv, 1.0, yv,
                                   op0=AL.mult, op1=AL.mult,
                                   accum_out=ssq[:, 0:1])
    nc.scalar.activation(_apf(sqb[:], [[1, 8064 - LD]], LD),
                         _apf(ybf[:], [[1, 8064 - LD]], LD), AF.Square,
                         accum_out=ssq[:, 1:2])

    stat2 = p.st.tile([128, 2], F32, tag="stat2")
    nc.vector.reduce_sum(stat2[:, 0:1], sacc[:], axis=mybir.AxisListType.X)
    nc.vector.tensor_tensor(stat2[:, 1:2], ssq[:, 0:1], ssq[:, 1:2], op=AL.add)

    st = Pools()
    st.p3, st.stat2, st.gsum = p3, stat2, tiles[7][0:64, 960:962]
    return st


def sample_phase2(nc, tc, p, c, st, y_out, n):
    """Stats tail + partition fold + finalize + output for sample n."""
    AL = mybir.AluOpType
    AF = mybir.ActivationFunctionType

    # group fold via SEL matmul into spare PSUM cols (never conv-written)
    nc.tensor.matmul(st.gsum, c.sel[:], st.stat2[:], start=True, stop=True)

    # partition fold: remap upper half down via DMA, then max
    p3b = p.pool.tile([64, 961], F16, tag="p3b")
    nc.sync.dma_start(p3b[:], st.p3[64:128, :])

    mv = p.st.tile([64, 2], F32, tag="mv")
    nc.vector.tensor_scalar(mv[:], st.gsum, 1.0 / NPIX, None, op0=AL.mult)
    msq = p.st.tile([64, 1], F32, tag="msq")
    nc.vector.tensor_tensor(msq[:], mv[:, 0:1], mv[:, 0:1], op=AL.mult)
    veps = p.st.tile([64, 1], F32, tag="veps")
    nc.vector.scalar_tensor_tensor(veps[:], mv[:, 1:2], EPS, msq[:],
                                   op0=AL.add, op1=AL.subtract)
    rv = p.st.tile([64, 1], F32, tag="rv")
    nc.vector.reciprocal(rv[:], veps[:])
    # aap = sqrt(rv * ws^2) = ws/std  (ws > 0, asserted host-side)
    aap = p.st.tile([64, 1], F32, tag="aap")
    nc.scalar.activation(aap[:], rv[:], AF.Sqrt, scale=c.ws2[:, 0:1])
    # bapneg = mean*aap - gb  (so fin = p4*aap - bapneg)
    bapneg = p.st.tile([64, 1], F32, tag="bapneg")
    nc.vector.scalar_tensor_tensor(bapneg[:], mv[:, 0:1], aap[:, 0:1],
                                   c.gb[:], op0=AL.mult, op1=AL.subtract)

    p4 = p.pool.tile([64, 961], F16, tag="p4")
    nc.vector.tensor_tensor(p4[:], st.p3[0:64, :], p3b[:], op=AL.max)

    # --- finalize: A*p - Bneg (f16, 4x), then split clamp-cast to f32 that
    # also reorders blocks u' = 8t+2g+c -> i = 8g+2t+c for a contiguous
    # out DMA (f32 writes stay 4B-aligned at odd 31-col offsets).
    fin = p.pool.tile([64, 961], F16, tag="fin")
    nc.vector.tensor_scalar(fin[:], p4[:], aap[:, 0:1], bapneg[:, 0:1],
                            op0=AL.mult, op1=AL.subtract)
    fin32 = p.pool.tile([64, 961], F32, tag="fin32")
    for sd, dd, off_s, off_d in (
            ([[248, 4], [62, 4], [1, 31]], [[62, 4], [248, 4], [1, 31]], 0, 0),
            ([[248, 3], [62, 4], [1, 31]], [[62, 3], [248, 4], [1, 31]], 31, 31),
            ([[62, 3], [1, 31]], [[248, 3], [1, 31]], 775, 217)):
        nc.vector.tensor_scalar(_apf(fin32[:], dd, off_d),
                                _apf(fin[:], sd, off_s),
                                0.0, 1.0, op0=AL.max, op1=AL.min)

    dst = _ap(y_out, [[961, 64], [1, 961]], n * 61504)
    nc.sync.dma_start(dst, fin32[:])


def build_kernel_nc(S, n_cores=8, repeat=1, use_for_i=False):
    nc = bacc.Bacc("TRN2", target_bir_lowering=False, debug=False,
                   num_devices=n_cores)
    x_in = nc.dram_tensor("x", [S * XELEM + 512], BF16,
                          kind="ExternalInput").ap()
    wq_in = nc.dram_tensor("wq", [128, 384], BF16, kind="ExternalInput").ap()
    sel_in = nc.dram_tensor("sel", [128, 64], F32, kind="ExternalInput").ap()
    bias_in = nc.dram_tensor("cbias", [128, 1], F32, kind="ExternalInput").ap()
    ws_in = nc.dram_tensor("ws", [64, 1], F32, kind="ExternalInput").ap()
    gb_in = nc.dram_tensor("gb", [64, 1], F32, kind="ExternalInput").ap()
    y_out = nc.dram_tensor("y", [S, 64, 31, 31], F32, kind="ExternalOutput").ap()
    with tile.TileContext(nc) as tc:
        with ExitStack() as ctx:
            p = build_pools(ctx, tc)
            c = load_consts(nc, p, wq_in, sel_in, bias_in, ws_in, gb_in)
            def pipelined_batch():
                pf = sample_prefetch(nc, p, x_in, 0)
                prev = None
                for n in range(S):
                    pf_next = (sample_prefetch(nc, p, x_in, n + 1)
                               if n + 1 < S else None)
                    if prev is not None:
                        sample_phase2(nc, tc, p, c, prev, y_out, n - 1)
                    prev = sample_phase1(nc, tc, p, c, pf, n)
                    pf = pf_next
                sample_phase2(nc, tc, p, c, prev, y_out, S - 1)

            if use_for_i and repeat > 1:
                with tc.For_i(0, repeat, 1):
                    pipelined_batch()
            else:
                for _ in range(repeat):
                    pipelined_batch()
    nc.compile()
    return nc


def make_consts(conv_w, conv_b, gn_w, gn_b, scale):
    """Host-side constant assembly."""
    import ml_dtypes
    conv_w = np.asarray(conv_w, dtype=np.float32)
    # w96[ic*4+kh+j, kw*128 + oc+64j] = conv_w[oc, ic, kh, kw]; quartered 4x
    w96 = np.zeros((32, 384), np.float32)
    oc = np.arange(64)
    for kw in range(3):
        for j in range(2):
            for ic in range(8):
                for kh in range(3):
                    w96[ic * 4 + kh + j, kw * 128 + oc + 64 * j] = \
                        conv_w[oc, ic, kh, kw]
    wq = np.tile(w96, (4, 1)).astype(ml_dtypes.bfloat16)
    sel = np.zeros((128, 64), np.float32)
    for j in range(2):
        for o in range(64):
            sel[o + 64 * j, (o // 4) * 4: (o // 4) * 4 + 4] = 1.0
    cbias = np.tile(np.asarray(conv_b, np.float32).reshape(64, 1), (2, 1))
    ws = (np.asarray(gn_w, np.float32).reshape(64) *
          np.asarray(scale, np.float32).reshape(64)).reshape(64, 1)
    assert np.all(ws > 0), "sqrt(rv*ws^2) trick needs ws > 0"
    gb = (np.asarray(gn_b, np.float32).reshape(64) *
          np.asarray(scale, np.float32).reshape(64)).reshape(64, 1)
    return dict(wq=wq, sel=sel, cbias=cbias.astype(np.float32),
                ws=(ws * ws).astype(np.float32), gb=gb.astype(np.float32))


# ---------------------------------------------------------------------------
# Harness entry point: full (unsharded) inputs -> full output.
# ---------------------------------------------------------------------------
N_CORES = 8
S_PER_CORE = 16
_NC_CACHE = {}


def _get_nc(repeat=1, use_for_i=False):
    key = (repeat, use_for_i)
    if key not in _NC_CACHE:
        _NC_CACHE[key] = build_kernel_nc(S_PER_CORE, n_cores=N_CORES,
                                         repeat=repeat, use_for_i=use_for_i)
    return _NC_CACHE[key]


def cast_x(x):
    import ml_dtypes
    return np.ascontiguousarray(np.asarray(x, dtype=np.float32)).astype(
        ml_dtypes.bfloat16)


def shard_x(xb, core):
    """Per-core padded flat x slice."""
    flat = xb[core * S_PER_CORE:(core + 1) * S_PER_CORE].reshape(-1)
    return np.concatenate([flat, np.zeros(512, dtype=flat.dtype)])


def kernel(x, conv_w, conv_b, gn_w, gn_b, scale):
    from concourse.bass_utils import run_bass_kernel_spmd
    xb = cast_x(x)
    consts = make_consts(conv_w, conv_b, gn_w, gn_b, scale)
    nc = _get_nc()
    in_maps = []
    for cc in range(N_CORES):
        m = dict(consts)
        m["x"] = shard_x(xb, cc)
        in_maps.append(m)
    res = run_bass_kernel_spmd(nc, in_maps, core_ids=list(range(N_CORES)))
    return np.concatenate([res.results[cc]["y"] for cc in range(N_CORES)],
                          axis=0)
